# revision 39
# baseline (speedup 1.0000x reference)
"""GQA attention layer (B=2, S=2048, D=4096, 32 Q heads / 8 KV heads, RoPE,
causal) on 8 Trainium2 NeuronCores, tensor-parallel over heads.

Each core owns 4 Q heads + 1 KV head and computes the whole layer for its
slice in ONE fused pass: per 1024-token half-batch it projects Q/K/V
(bf16 operands, fp32 PSUM accumulation), applies RoPE straight out of
PSUM on the vector engine (swap-free half-partition formulation), runs
causal attention out of SBUF-resident K/V, and emits a streamed output
projection deferred by one half so normalization latency never stalls
the PE.  The host sums the 8 partial outputs (bf16 on the wire).

Key layout choices:
 - activations feature-major [feature_partition, token_free]; every
   matmul contracts over the partition dim.
 - projection PSUM packs two 256-wide outputs per 2KB bank using the
   per-element has_written semantics (single start=True per bank).
 - V is transposed to token-major on the PE (deferred batch of
   transposes per half, packed 2 per PSUM bank).
 - softmax denominators: exp tiles accumulated on DVE, reduced across
   partitions + broadcast in one gpsimd partition_all_reduce.
"""

import os
import sys
import types
from contextlib import ExitStack

import numpy as np
import ml_dtypes

import concourse.bass as bass
import concourse.tile as tile
from concourse import bacc
from concourse import mybir
from concourse import bass_utils
from concourse.bass_utils import run_bass_kernel_spmd

# Optional NTFF profiling support under axon (trimmed image lacks
# antenv.axon_hooks); harmless when unavailable.
try:
    import antenv  # noqa: F401
    from trn_agent_boot.trn_boot import _ntff_profile_via_ctypes

    if "antenv.axon_hooks" not in sys.modules:
        _hooks_mod = types.ModuleType("antenv.axon_hooks")
        _hook = _ntff_profile_via_ctypes("/opt/axon/libaxon_pjrt.so")
        _hooks_mod.get_axon_ntff_profile_hook = lambda: _hook
        _hooks_mod.set_axon_ntff_profile_hook = lambda h: None
        sys.modules["antenv.axon_hooks"] = _hooks_mod
    bass_utils.upload_artifacts = lambda tmpdir: "local://skipped"
except Exception:
    pass

F32 = mybir.dt.float32
F32R = mybir.dt.float32r
BF16 = mybir.dt.bfloat16
EXP = mybir.ActivationFunctionType.Exp

B, S, D = 2, 2048, 4096
NH, NKV, HD = 32, 8, 128
T = B * S
N_CORES = 8
QH = NH // N_CORES              # 4 local q heads
FL = QH * HD                    # 512 local q features
SCALE = 1.0 / float(np.sqrt(HD))
NEG = -1.0e30

NW = 256                        # tokens per projection group
HALF = 1024                     # tokens per fused pipeline stage
NG = HALF // NW                 # 4 proj groups per half
QB = 512                        # q-block width in attention
DKD = D // 128                  # 32 contraction chunks
NJB = 4                         # x/wq chunk-groups (8 k-chunks each)
NDG = D // QB                   # 8 output-projection column blocks


def _build_program():
    nc = bacc.Bacc("TRN2", target_bir_lowering=False, debug=False,
                   num_devices=N_CORES)

    # Host-pretiled inputs (see kernel() for exact layouts).
    x4 = nc.dram_tensor("x4", [16 * NJB * 128, 8 * NW], BF16,
                        kind="ExternalInput").ap()
    wq4 = nc.dram_tensor("wq4", [128, DKD * FL], BF16, kind="ExternalInput").ap()
    wk4 = nc.dram_tensor("wk4", [128, DKD * HD], BF16, kind="ExternalInput").ap()
    wv4 = nc.dram_tensor("wv4", [128, DKD * HD], BF16, kind="ExternalInput").ap()
    wot = nc.dram_tensor("wot", [NDG * 128, QH * QB], F32R,
                         kind="ExternalInput").ap()
    ropc = nc.dram_tensor("ropc", [HD, S], F32, kind="ExternalInput").ap()
    rops = nc.dram_tensor("rops", [HD, S], F32, kind="ExternalInput").ap()
    idin = nc.dram_tensor("idin", [128, 128], F32R, kind="ExternalInput").ap()
    onesin = nc.dram_tensor("onesin", [128, 1], F32R, kind="ExternalInput").ap()
    maskt = [nc.dram_tensor(f"maskt{v}", [128, QB], F32R,
                            kind="ExternalInput").ap() for v in range(4)]
    # Output: y_t[tg, dg] = y[tg*128:(tg+1)*128, dg*512:(dg+1)*512] in bf16.
    y_t = nc.dram_tensor("y_t", [(T // 128) * NDG * 128, QB], BF16,
                         kind="ExternalOutput").ap()

    with tile.TileContext(nc) as tc, ExitStack() as ctx:
        # ------------------------------------------------------------------
        # Persistent SBUF tiles
        # ------------------------------------------------------------------
        # Weight loads first (wq chunk-group 0 before everything else so the
        # first projection matmuls can start within a few microseconds; the
        # remaining const loads are emitted later to keep early DMA
        # semaphore lanes free — lanes recycle round-robin and a DMA behind
        # a busy lane inherits its predecessor's completion wait).
        wpool = ctx.enter_context(tc.tile_pool(name="weights", bufs=1))
        wq_sb = wpool.tile([128, DKD * FL], BF16, tag="wq")
        nc.sync.dma_start(wq_sb[:, 0:4 * FL], wq4[:, 0:4 * FL])
        wk_sb = wpool.tile([128, DKD * HD], BF16, tag="wk")
        wv_sb = wpool.tile([128, DKD * HD], BF16, tag="wv")

        const = ctx.enter_context(tc.tile_pool(name="const", bufs=1))
        ident = const.tile([128, 128], F32R)
        ones_t = const.tile([128, 1], F32R, tag="ones")
        mtv = [const.tile([128, QB], F32R, tag=f"mtv{v}", name=f"mtv{v}")
               for v in range(4)]

        def emit_late_weight_loads():
            for jb in range(1, NJB):
                nc.sync.dma_start(
                    wq_sb[:, jb * 8 * FL:(jb + 1) * 8 * FL],
                    wq4[:, jb * 8 * FL:(jb + 1) * 8 * FL])

        def emit_const_loads():
            nc.sync.dma_start(ident[:], idin)
            nc.sync.dma_start(ones_t[:], onesin)
            for v in range(4):
                nc.sync.dma_start(mtv[v][:], maskt[v])

        rcpool = ctx.enter_context(tc.tile_pool(name="ropec", bufs=1))
        cos_s = rcpool.tile([HD, HALF], F32, tag="cos")
        sin_s = rcpool.tile([HD, HALF], F32, tag="sin")

        apool = ctx.enter_context(tc.tile_pool(name="acts", bufs=1))
        q_half = [apool.tile([128, HALF], F32R, tag=f"qh{m}", name=f"qh{m}")
                  for m in range(QH)]
        kT = apool.tile([128, S], F32R, tag="kT")
        V_b = apool.tile([128, S], F32R, tag="V_b")

        attpool = ctx.enter_context(tc.tile_pool(name="att", bufs=2))
        wopool = ctx.enter_context(tc.tile_pool(name="wostream", bufs=2))
        xpool = ctx.enter_context(tc.tile_pool(name="xin", bufs=3))
        ptpool = ctx.enter_context(tc.tile_pool(name="pt", bufs=4))
        accpool = ctx.enter_context(tc.tile_pool(name="acc", bufs=2))
        smpool = ctx.enter_context(tc.tile_pool(name="sums", bufs=2))
        aupool = ctx.enter_context(tc.tile_pool(name="attun", bufs=2))
        yspool = ctx.enter_context(tc.tile_pool(name="ystage", bufs=8))
        vstpool = ctx.enter_context(tc.tile_pool(name="vst", bufs=4))
        rtpool = ctx.enter_context(tc.tile_pool(name="ropetmp", bufs=4))
        ypsum = ctx.enter_context(tc.tile_pool(name="yps", bufs=2, space="PSUM"))

        def rope_evict(ps, out, g):
            """RoPE on the even/odd-split feature layout, swap-free:
            out[0:64]  = ps[0:64]*cos + ps[64:128]*(-sin)
            out[64:128]= ps[64:128]*cos + ps[0:64]*(+sin)
            (cos_s rows duplicated; sin_s rows = [-sin; sin]).
            Reads ps (PSUM) directly on the vector engine."""
            c0 = g * NW
            tmp = rtpool.tile([128, NW], F32, tag="rt")
            # the four PSUM-reading muls stay on DVE; the two adds go to
            # the (otherwise idle) gpsimd engine to keep DVE off the
            # critical path
            nc.vector.tensor_mul(tmp[0:64, :], ps[64:128, :],
                                 sin_s[0:64, c0:c0 + NW])
            nc.vector.tensor_mul(out[0:64, :], ps[0:64, :],
                                 cos_s[0:64, c0:c0 + NW])
            nc.vector.tensor_mul(tmp[64:128, :], ps[0:64, :],
                                 sin_s[64:128, c0:c0 + NW])
            nc.vector.tensor_mul(out[64:128, :], ps[64:128, :],
                                 cos_s[64:128, c0:c0 + NW])
            nc.gpsimd.tensor_add(out[0:64, :], out[0:64, :], tmp[0:64, :])
            nc.gpsimd.tensor_add(out[64:128, :], out[64:128, :],
                                 tmp[64:128, :])

        def emit_wo(att_h, b, hb, dgs):
            """Output projection for one 1024-token half: y-slice =
            att_h (4x[128,1024] feature-major) contracted with streamed
            wo column blocks."""
            tg0 = b * (S // 128) + hb * (HALF // 128)
            pending_y = None
            for dg in dgs:
                wod = wopool.tile([128, QH * QB], F32R, tag="wod")
                nc.sync.dma_start(wod[:], wot[dg * 128:(dg + 1) * 128, :])
                for tcx in range(HALF // 128):
                    yp = ypsum.tile([128, QB], F32, tag="yp")
                    for f in range(QH):
                        nc.tensor.matmul(
                            yp[:], att_h[f][:, tcx * 128:(tcx + 1) * 128],
                            wod[:, f * QB:(f + 1) * QB],
                            start=(f == 0), stop=(f == QH - 1))
                    # PSUM eviction alternates engines so neither the DVE
                    # nor the ACT queue serializes a full half's worth of
                    # casts ahead of the attention eviction chain
                    ysb = yspool.tile([128, QB], BF16, tag="ysb")
                    if (dg + tcx) % 2 == 0:
                        nc.vector.tensor_copy(ysb[:], yp[:])
                    else:
                        nc.scalar.copy(ysb[:], yp[:])
                    # the output DMA issues one tile late: by then its cast
                    # has completed, so the sync ring's head never blocks
                    # (a blocked head delays every later xg/wod prefetch)
                    if pending_y is not None:
                        nc.sync.dma_start(*pending_y)
                    tg = tg0 + tcx
                    pending_y = (
                        y_t[(tg * NDG + dg) * 128:(tg * NDG + dg + 1) * 128, :],
                        ysb[:])
            nc.sync.dma_start(*pending_y)

        pending = None
        for b in range(B):
            for hb in range(2):
                base = hb * HALF            # position within batch
                first_half = (b == 0 and hb == 0)
                if not first_half:
                    nc.sync.dma_start(cos_s[:], ropc[:, base:base + HALF])
                    nc.sync.dma_start(sin_s[:], rops[:, base:base + HALF])

                # ----------------------------------------------------------
                # QKV projection + RoPE for this half
                # ----------------------------------------------------------
                with tc.tile_pool(name="projps", bufs=1, space="PSUM") as pps:
                    def emit_vt(g, vst):
                        """V transpose to token-major, 2 per PSUM bank."""
                        vtp = pps.tile([128, NW], F32R, tag="vtp", bufs=1)
                        nc.tensor.transpose(
                            vtp[:, 0:128], vst[:, 0:128], ident[:])
                        nc.tensor.transpose(
                            vtp[:, 128:256], vst[:, 128:256], ident[:])
                        nc.vector.tensor_copy(
                            V_b[:, base + g * NW:base + (g + 1) * NW], vtp[:])

                    vt_pending = None
                    for g in range(NG):
                        gg = (b * S + base) // NW + g   # global 256-tok group
                        qab = pps.tile([128, 2 * NW], F32, tag="qab", bufs=2)
                        qcd = pps.tile([128, 2 * NW], F32, tag="qcd", bufs=2)
                        kv = pps.tile([128, 2 * NW], F32, tag="kv", bufs=1)
                        for jb in range(NJB):
                            xg = xpool.tile([128, 8 * NW], BF16, tag="xg")
                            r0 = (gg * NJB + jb) * 128
                            if first_half and g == 0 and jb == 0:
                                # staged startup: load the first x block in
                                # halves so the very first matmul gates on
                                # ~1MB of DMA, not ~6MB
                                nc.sync.dma_start(xg[:, 0:4 * NW],
                                                  x4[r0:r0 + 128, 0:4 * NW])
                                nc.sync.dma_start(
                                    wq_sb[:, 4 * FL:8 * FL],
                                    wq4[:, 4 * FL:8 * FL])
                                nc.sync.dma_start(xg[:, 4 * NW:8 * NW],
                                                  x4[r0:r0 + 128, 4 * NW:8 * NW])
                                nc.sync.dma_start(wk_sb[:], wk4)
                                nc.sync.dma_start(wv_sb[:], wv4)
                            else:
                                nc.sync.dma_start(xg[:], x4[r0:r0 + 128, :])
                            if first_half and g == 0 and jb == 1:
                                emit_late_weight_loads()
                            if first_half and g == 0 and jb == 2:
                                # rope tables only needed at this group's
                                # evictions; keep the first x tile's DMA
                                # bandwidth share high
                                nc.sync.dma_start(
                                    cos_s[:], ropc[:, base:base + HALF])
                                nc.sync.dma_start(
                                    sin_s[:], rops[:, base:base + HALF])
                            # q matmuls first, kv last: the kv bank is
                            # single-buffered, so its previous-group RoPE /
                            # copy reads get ~3.4us of q-matmul cover before
                            # the next write touches the bank.
                            for c in range(8):
                                k = jb * 8 + c
                                xs = xg[:, c * NW:(c + 1) * NW]
                                st = (k == 0)
                                sp = (k == DKD - 1)
                                nc.tensor.matmul(
                                    qab[:, 0:NW],
                                    wq_sb[:, k * FL:k * FL + 128], xs,
                                    start=st, stop=False)
                                nc.tensor.matmul(
                                    qab[:, NW:2 * NW],
                                    wq_sb[:, k * FL + 128:k * FL + 256], xs,
                                    start=False, stop=sp)
                                nc.tensor.matmul(
                                    qcd[:, 0:NW],
                                    wq_sb[:, k * FL + 256:k * FL + 384], xs,
                                    start=st, stop=False)
                                nc.tensor.matmul(
                                    qcd[:, NW:2 * NW],
                                    wq_sb[:, k * FL + 384:k * FL + 512], xs,
                                    start=False, stop=sp)
                            for c in range(8):
                                k = jb * 8 + c
                                xs = xg[:, c * NW:(c + 1) * NW]
                                st = (k == 0)
                                sp = (k == DKD - 1)
                                nc.tensor.matmul(
                                    kv[:, 0:NW],
                                    wk_sb[:, k * HD:(k + 1) * HD], xs,
                                    start=st, stop=False)
                                nc.tensor.matmul(
                                    kv[:, NW:2 * NW],
                                    wv_sb[:, k * HD:(k + 1) * HD], xs,
                                    start=False, stop=sp)
                        # evictions: k rope + v copy first (kv is bufs=1) so
                        # the next group's kv matmuls aren't gated on them —
                        # except for the LAST group, where the q ropes go
                        # first: the attention phase reuses these PSUM banks
                        # and its first score matmuls wait on the q reads.
                        vst = vstpool.tile([128, NW], F32R, tag="vst")
                        last_g = (g == NG - 1)
                        if not last_g:
                            rope_evict(kv[:, 0:NW],
                                       kT[:, base + g * NW:base + (g + 1) * NW],
                                       g)
                            nc.scalar.copy(vst[:], kv[:, NW:2 * NW])
                        # previous group's V transpose here: its DVE copy
                        # overlaps this group's matmuls (vtp is bufs=1)
                        if vt_pending is not None:
                            emit_vt(*vt_pending)
                        vt_pending = (g, vst)
                        rope_evict(qab[:, 0:NW],
                                   q_half[0][:, g * NW:(g + 1) * NW], g)
                        rope_evict(qab[:, NW:2 * NW],
                                   q_half[1][:, g * NW:(g + 1) * NW], g)
                        rope_evict(qcd[:, 0:NW],
                                   q_half[2][:, g * NW:(g + 1) * NW], g)
                        rope_evict(qcd[:, NW:2 * NW],
                                   q_half[3][:, g * NW:(g + 1) * NW], g)
                        if last_g:
                            rope_evict(kv[:, 0:NW],
                                       kT[:, base + g * NW:base + (g + 1) * NW],
                                       g)
                            nc.scalar.copy(vst[:], kv[:, NW:2 * NW])
                        if b == 0 and hb == 0 and g == 0:
                            emit_const_loads()
                    emit_vt(*vt_pending)

                # first wo blocks of the previous half right at the
                # proj->attn boundary: ready PE work that covers the PSUM
                # bank WAR waits on the trailing RoPE reads
                if pending is not None:
                    emit_wo(*pending, dgs=range(0, 3))

                # ----------------------------------------------------------
                # Attention for this half (q blocks of 512)
                # ----------------------------------------------------------
                att_h = [attpool.tile([128, HALF], F32R, tag=f"at{f}",
                                      name=f"at{f}") for f in range(QH)]
                with tc.tile_pool(name="attnps", bufs=1, space="PSUM") as aps:
                    for qb in range(HALF // QB):
                        q0 = qb * QB
                        nkt = (base + q0 + QB) // 128
                        for h in range(QH):
                            avp = aps.tile([128, QB], F32, tag="avp", bufs=3)
                            smp = aps.tile([1, QB], F32, tag="smp", bufs=1)
                            acc = accpool.tile([128, QB], F32R, tag="acc")
                            for ktc in range(nkt):
                                stp = aps.tile([128, QB], F32, tag="stp",
                                               bufs=2)
                                diag = ktc >= nkt - 4
                                nc.tensor.matmul(
                                    stp[:], kT[:, ktc * 128:(ktc + 1) * 128],
                                    q_half[h][:, q0:q0 + QB],
                                    start=True, stop=not diag)
                                if diag:
                                    # causal mask added on the PE: I.T @ M
                                    # accumulates M into the score bank,
                                    # keeping the chunk chain PE->ACT only.
                                    # chunk v only masks q columns < (v+1)*128
                                    # (the rest of the mask slice is zeros)
                                    v = ktc - (nkt - 4)
                                    mw = (v + 1) * 128
                                    nc.tensor.matmul(
                                        stp[:, 0:mw], ident[:],
                                        mtv[v][:, 0:mw],
                                        start=False, stop=True)
                                pt = ptpool.tile([128, QB], F32R, tag="pt")
                                nc.scalar.activation(pt[:], stp[:], EXP,
                                                     scale=SCALE)
                                nc.tensor.matmul(
                                    avp[:], V_b[:, ktc * 128:(ktc + 1) * 128],
                                    pt[:], start=(ktc == 0),
                                    stop=(ktc == nkt - 1))
                                # exp tiles accumulate on DVE; one ones-matmul
                                # per head then reduces over partitions (vs a
                                # third matmul on every chunk)
                                if ktc == 0:
                                    nc.vector.tensor_copy(acc[:], pt[:])
                                else:
                                    nc.vector.tensor_add(acc[:], acc[:], pt[:])
                            nc.tensor.matmul(smp[:], ones_t[:, 0:1], acc[:],
                                             start=True, stop=True)
                            # evictions alternate ACT/DVE by head so a
                            # backlog on either queue can't stall the avp
                            # bank rotation two heads later
                            att_un = aupool.tile([128, QB], F32R, tag="au")
                            s_sb = smpool.tile([1, QB], F32, tag="ssb")
                            if h % 2 == 0:
                                nc.scalar.copy(att_un[:], avp[:])
                                nc.vector.tensor_copy(s_sb[:], smp[:])
                            else:
                                nc.vector.tensor_copy(att_un[:], avp[:])
                                nc.scalar.copy(s_sb[:], smp[:])
                            r_sb = smpool.tile([1, QB], F32, tag="rsb")
                            nc.vector.reciprocal_approx_fast(r_sb[:], s_sb[:])
                            r_bc = accpool.tile([128, QB], F32, tag="rbc")
                            nc.gpsimd.partition_broadcast(r_bc[:], r_sb[:])
                            nc.vector.tensor_mul(
                                att_h[h][:, q0:q0 + QB], att_un[:], r_bc[:])
                # rest of the previous half's output projection: emitted
                # after this half's attention; the scheduler interleaves
                # its PE work into attention's dependency stalls
                if pending is not None:
                    emit_wo(*pending, dgs=range(3, NDG))
                pending = (att_h, b, hb)
        emit_wo(*pending, dgs=range(NDG))
    nc.compile()
    return nc


_program = None


def _get_program():
    global _program
    if _program is None:
        _program = _build_program()
    return _program


def kernel(**inputs) -> np.ndarray:
    x = np.asarray(inputs["x"], dtype=np.float32)
    wq = np.asarray(inputs["wq"], dtype=np.float32)
    wk = np.asarray(inputs["wk"], dtype=np.float32)
    wv = np.asarray(inputs["wv"], dtype=np.float32)
    wo = np.asarray(inputs["wo"], dtype=np.float32)
    cos = np.asarray(inputs["freqs_cos"], dtype=np.float32)
    sin = np.asarray(inputs["freqs_sin"], dtype=np.float32)
    mask = np.asarray(inputs["mask"], dtype=np.float32)
    start_pos = int(np.asarray(inputs.get("start_pos", 0)))
    assert start_pos == 0, "kernel specialized for start_pos == 0"

    # Even/odd RoPE pair split within each head's 128 features.
    perm = np.concatenate([np.arange(0, HD, 2), np.arange(1, HD, 2)])

    # x tiled: x4[gg, jb] rows = [128, 8*256] where row p, col c*256+w =
    # x_token[gg*256 + w, (jb*8+c)*128 + p]
    xT = x.reshape(T, D).T                              # [D, T]
    x4 = np.ascontiguousarray(
        xT.reshape(NJB, 8, 128, 16, NW).transpose(3, 0, 2, 1, 4)
        .reshape(16 * NJB * 128, 8 * NW)).astype(ml_dtypes.bfloat16)

    cosT = cos.T                                        # [64, S]
    sinT = sin.T
    ropc = np.ascontiguousarray(np.concatenate([cosT, cosT], axis=0))
    rops = np.ascontiguousarray(np.concatenate([-sinT, sinT], axis=0))
    masktv = [np.ascontiguousarray(
        np.maximum(mask[:QB, v * 128:(v + 1) * 128], NEG)
        .astype(np.float32).T) for v in range(4)]

    in_maps = []
    for c in range(N_CORES):
        wq_c = (wq[c * FL:(c + 1) * FL].reshape(QH, HD, D)[:, perm, :]
                .reshape(FL, D))
        wk_c = wk[c * HD:(c + 1) * HD][perm, :]
        wv_c = wv[c * HD:(c + 1) * HD]
        wo_c = wo[:, c * FL:(c + 1) * FL]
        # wq4[p, k*512 + f] = wq_c[f, k*128+p]  (k-chunk-major, bf16)
        wq4 = np.ascontiguousarray(
            wq_c.T.reshape(DKD, 128, FL).transpose(1, 0, 2)
            .reshape(128, DKD * FL)).astype(ml_dtypes.bfloat16)
        wk4 = np.ascontiguousarray(
            wk_c.T.reshape(DKD, 128, HD).transpose(1, 0, 2)
            .reshape(128, DKD * HD)).astype(ml_dtypes.bfloat16)
        wv4 = np.ascontiguousarray(
            wv_c.T.reshape(DKD, 128, HD).transpose(1, 0, 2)
            .reshape(128, DKD * HD)).astype(ml_dtypes.bfloat16)
        # wot[dg*128+p, f*512+c] = wo_c[dg*512+c, f*128+p]
        wot = np.ascontiguousarray(
            wo_c.T.reshape(QH, 128, NDG, QB).transpose(2, 1, 0, 3)
            .reshape(NDG * 128, QH * QB))
        in_maps.append({
            "x4": x4,
            "wq4": wq4,
            "wk4": wk4,
            "wv4": wv4,
            "wot": wot,
            "ropc": ropc,
            "rops": rops,
            "idin": np.eye(128, dtype=np.float32),
            "onesin": np.ones((128, 1), dtype=np.float32),
            "maskt0": masktv[0],
            "maskt1": masktv[1],
            "maskt2": masktv[2],
            "maskt3": masktv[3],
        })

    nc = _get_program()
    trace = bool(int(os.environ.get("GQA_TRACE", "0")))
    kwargs = {}
    if trace:
        tmpdir = os.environ.get("GQA_TRACE_DIR") or None
        kwargs = dict(trace=True, tmpdir=tmpdir, trace_cores=[0])
    res = run_bass_kernel_spmd(nc, in_maps, list(range(N_CORES)), **kwargs)
    kernel.last_results = res

    acc = np.zeros((T // 128, 128, D), dtype=np.float64)
    for c in range(N_CORES):
        yt = np.asarray(res.results[c]["y_t"], dtype=np.float64)
        acc += yt.reshape(T // 128, NDG, 128, QB).transpose(0, 2, 1, 3) \
                 .reshape(T // 128, 128, D)
    return acc.astype(np.float32).reshape(B, S, D)


# revision 40
# speedup vs baseline: 1.0216x; 1.0216x over previous
"""GQA attention layer (B=2, S=2048, D=4096, 32 Q heads / 8 KV heads, RoPE,
causal) on 8 Trainium2 NeuronCores, tensor-parallel over heads.

Each core owns 4 Q heads + 1 KV head and computes the whole layer for its
slice in ONE fused pass: per 1024-token half-batch it projects Q/K/V
(bf16 operands, fp32 PSUM accumulation), applies RoPE straight out of
PSUM on the vector engine (swap-free half-partition formulation), runs
causal attention out of SBUF-resident K/V, and emits a streamed output
projection deferred by one half so normalization latency never stalls
the PE.  The host sums the 8 partial outputs (bf16 on the wire).

Key layout choices:
 - activations feature-major [feature_partition, token_free]; every
   matmul contracts over the partition dim.
 - projection PSUM packs two 256-wide outputs per 2KB bank using the
   per-element has_written semantics (single start=True per bank).
 - V is transposed to token-major on the PE (deferred batch of
   transposes per half, packed 2 per PSUM bank).
 - softmax denominators: exp tiles accumulated on DVE, reduced across
   partitions + broadcast in one gpsimd partition_all_reduce.
"""

import os
import sys
import types
from contextlib import ExitStack

import numpy as np
import ml_dtypes

import concourse.bass as bass
import concourse.tile as tile
from concourse import bacc
from concourse import mybir
from concourse import bass_utils
from concourse.bass_utils import run_bass_kernel_spmd

# Optional NTFF profiling support under axon (trimmed image lacks
# antenv.axon_hooks); harmless when unavailable.
try:
    import antenv  # noqa: F401
    from trn_agent_boot.trn_boot import _ntff_profile_via_ctypes

    if "antenv.axon_hooks" not in sys.modules:
        _hooks_mod = types.ModuleType("antenv.axon_hooks")
        _hook = _ntff_profile_via_ctypes("/opt/axon/libaxon_pjrt.so")
        _hooks_mod.get_axon_ntff_profile_hook = lambda: _hook
        _hooks_mod.set_axon_ntff_profile_hook = lambda h: None
        sys.modules["antenv.axon_hooks"] = _hooks_mod
    bass_utils.upload_artifacts = lambda tmpdir: "local://skipped"
except Exception:
    pass

F32 = mybir.dt.float32
F32R = mybir.dt.float32r
BF16 = mybir.dt.bfloat16
EXP = mybir.ActivationFunctionType.Exp

B, S, D = 2, 2048, 4096
NH, NKV, HD = 32, 8, 128
T = B * S
N_CORES = 8
QH = NH // N_CORES              # 4 local q heads
FL = QH * HD                    # 512 local q features
SCALE = 1.0 / float(np.sqrt(HD))
NEG = -1.0e30

NW = 256                        # tokens per projection group
HALF = 1024                     # tokens per fused pipeline stage
NG = HALF // NW                 # 4 proj groups per half
QB = 512                        # q-block width in attention
DKD = D // 128                  # 32 contraction chunks
NJB = 4                         # x/wq chunk-groups (8 k-chunks each)
NDG = D // QB                   # 8 output-projection column blocks


def _build_program():
    nc = bacc.Bacc("TRN2", target_bir_lowering=False, debug=False,
                   num_devices=N_CORES)

    # Host-pretiled inputs (see kernel() for exact layouts).
    x4 = nc.dram_tensor("x4", [16 * NJB * 128, 8 * NW], BF16,
                        kind="ExternalInput").ap()
    wq4 = nc.dram_tensor("wq4", [128, DKD * FL], BF16, kind="ExternalInput").ap()
    wk4 = nc.dram_tensor("wk4", [128, DKD * HD], BF16, kind="ExternalInput").ap()
    wv4 = nc.dram_tensor("wv4", [128, DKD * HD], BF16, kind="ExternalInput").ap()
    wot = nc.dram_tensor("wot", [NDG * 128, QH * QB], F32R,
                         kind="ExternalInput").ap()
    ropc = nc.dram_tensor("ropc", [HD, S], F32, kind="ExternalInput").ap()
    rops = nc.dram_tensor("rops", [HD, S], F32, kind="ExternalInput").ap()
    idin = nc.dram_tensor("idin", [128, 128], F32R, kind="ExternalInput").ap()
    onesin = nc.dram_tensor("onesin", [128, 1], F32R, kind="ExternalInput").ap()
    maskt = [nc.dram_tensor(f"maskt{v}", [128, QB], F32R,
                            kind="ExternalInput").ap() for v in range(4)]
    # Output: y_t[tg, dg] = y[tg*128:(tg+1)*128, dg*512:(dg+1)*512] in bf16.
    y_t = nc.dram_tensor("y_t", [(T // 128) * NDG * 128, QB], BF16,
                         kind="ExternalOutput").ap()

    with tile.TileContext(nc) as tc, ExitStack() as ctx:
        # ------------------------------------------------------------------
        # Persistent SBUF tiles
        # ------------------------------------------------------------------
        # Weight loads first (wq chunk-group 0 before everything else so the
        # first projection matmuls can start within a few microseconds; the
        # remaining const loads are emitted later to keep early DMA
        # semaphore lanes free — lanes recycle round-robin and a DMA behind
        # a busy lane inherits its predecessor's completion wait).
        wpool = ctx.enter_context(tc.tile_pool(name="weights", bufs=1))
        wq_sb = wpool.tile([128, DKD * FL], BF16, tag="wq")
        nc.sync.dma_start(wq_sb[:, 0:4 * FL], wq4[:, 0:4 * FL])
        wk_sb = wpool.tile([128, DKD * HD], BF16, tag="wk")
        wv_sb = wpool.tile([128, DKD * HD], BF16, tag="wv")

        const = ctx.enter_context(tc.tile_pool(name="const", bufs=1))
        ident = const.tile([128, 128], F32R)
        ones_t = const.tile([128, 1], F32R, tag="ones")
        mtv = [const.tile([128, QB], F32R, tag=f"mtv{v}", name=f"mtv{v}")
               for v in range(4)]

        def emit_late_weight_loads():
            for jb in range(1, NJB):
                nc.sync.dma_start(
                    wq_sb[:, jb * 8 * FL:(jb + 1) * 8 * FL],
                    wq4[:, jb * 8 * FL:(jb + 1) * 8 * FL])

        def emit_const_loads():
            nc.sync.dma_start(ident[:], idin)
            nc.sync.dma_start(ones_t[:], onesin)
            for v in range(4):
                nc.sync.dma_start(mtv[v][:], maskt[v])

        rcpool = ctx.enter_context(tc.tile_pool(name="ropec", bufs=1))
        cos_s = rcpool.tile([HD, HALF], F32, tag="cos")
        sin_s = rcpool.tile([HD, HALF], F32, tag="sin")

        apool = ctx.enter_context(tc.tile_pool(name="acts", bufs=1))
        q_half = [apool.tile([128, HALF], F32R, tag=f"qh{m}", name=f"qh{m}")
                  for m in range(QH)]
        kT = apool.tile([128, S], F32R, tag="kT")
        V_b = apool.tile([128, S], F32R, tag="V_b")

        attpool = ctx.enter_context(tc.tile_pool(name="att", bufs=2))
        wopool = ctx.enter_context(tc.tile_pool(name="wostream", bufs=2))
        xpool = ctx.enter_context(tc.tile_pool(name="xin", bufs=3))
        ptpool = ctx.enter_context(tc.tile_pool(name="pt", bufs=4))
        accpool = ctx.enter_context(tc.tile_pool(name="acc", bufs=2))
        smpool = ctx.enter_context(tc.tile_pool(name="sums", bufs=2))
        aupool = ctx.enter_context(tc.tile_pool(name="attun", bufs=2))
        yspool = ctx.enter_context(tc.tile_pool(name="ystage", bufs=8))
        vstpool = ctx.enter_context(tc.tile_pool(name="vst", bufs=4))
        rtpool = ctx.enter_context(tc.tile_pool(name="ropetmp", bufs=4))
        ypsum = ctx.enter_context(tc.tile_pool(name="yps", bufs=2, space="PSUM"))

        def rope_evict(ps, out, g):
            """RoPE on the even/odd-split feature layout, swap-free:
            out[0:64]  = ps[0:64]*cos + ps[64:128]*(-sin)
            out[64:128]= ps[64:128]*cos + ps[0:64]*(+sin)
            (cos_s rows duplicated; sin_s rows = [-sin; sin]).
            Reads ps (PSUM) directly on the vector engine."""
            c0 = g * NW
            tmp = rtpool.tile([128, NW], F32, tag="rt")
            # the four PSUM-reading muls stay on DVE; the two adds go to
            # the (otherwise idle) gpsimd engine to keep DVE off the
            # critical path
            nc.vector.tensor_mul(tmp[0:64, :], ps[64:128, :],
                                 sin_s[0:64, c0:c0 + NW])
            nc.vector.tensor_mul(out[0:64, :], ps[0:64, :],
                                 cos_s[0:64, c0:c0 + NW])
            nc.vector.tensor_mul(tmp[64:128, :], ps[0:64, :],
                                 sin_s[64:128, c0:c0 + NW])
            nc.vector.tensor_mul(out[64:128, :], ps[64:128, :],
                                 cos_s[64:128, c0:c0 + NW])
            nc.gpsimd.tensor_add(out[0:64, :], out[0:64, :], tmp[0:64, :])
            nc.gpsimd.tensor_add(out[64:128, :], out[64:128, :],
                                 tmp[64:128, :])

        def emit_wo(att_h, b, hb):
            """Output projection for one 1024-token half: y-slice =
            att_h (4x[128,1024] feature-major) contracted with streamed
            wo column blocks."""
            tg0 = b * (S // 128) + hb * (HALF // 128)
            pending_y = None
            for dg in range(NDG):
                wod = wopool.tile([128, QH * QB], F32R, tag="wod")
                nc.sync.dma_start(wod[:], wot[dg * 128:(dg + 1) * 128, :])
                for tcx in range(HALF // 128):
                    yp = ypsum.tile([128, QB], F32, tag="yp")
                    for f in range(QH):
                        nc.tensor.matmul(
                            yp[:], att_h[f][:, tcx * 128:(tcx + 1) * 128],
                            wod[:, f * QB:(f + 1) * QB],
                            start=(f == 0), stop=(f == QH - 1))
                    # PSUM eviction alternates engines so neither the DVE
                    # nor the ACT queue serializes a full half's worth of
                    # casts ahead of the attention eviction chain
                    ysb = yspool.tile([128, QB], BF16, tag="ysb")
                    if (dg + tcx) % 2 == 0:
                        nc.vector.tensor_copy(ysb[:], yp[:])
                    else:
                        nc.scalar.copy(ysb[:], yp[:])
                    # the output DMA issues one tile late: by then its cast
                    # has completed, so the sync ring's head never blocks
                    # (a blocked head delays every later xg/wod prefetch)
                    if pending_y is not None:
                        nc.sync.dma_start(*pending_y)
                    tg = tg0 + tcx
                    pending_y = (
                        y_t[(tg * NDG + dg) * 128:(tg * NDG + dg + 1) * 128, :],
                        ysb[:])
            nc.sync.dma_start(*pending_y)

        pending = None
        for b in range(B):
            for hb in range(2):
                base = hb * HALF            # position within batch
                first_half = (b == 0 and hb == 0)
                if not first_half:
                    nc.sync.dma_start(cos_s[:], ropc[:, base:base + HALF])
                    nc.sync.dma_start(sin_s[:], rops[:, base:base + HALF])

                # ----------------------------------------------------------
                # QKV projection + RoPE for this half
                # ----------------------------------------------------------
                with tc.tile_pool(name="projps", bufs=1, space="PSUM") as pps:
                    def emit_vt(g, vst):
                        """V transpose to token-major, 2 per PSUM bank."""
                        vtp = pps.tile([128, NW], F32R, tag="vtp", bufs=1)
                        nc.tensor.transpose(
                            vtp[:, 0:128], vst[:, 0:128], ident[:])
                        nc.tensor.transpose(
                            vtp[:, 128:256], vst[:, 128:256], ident[:])
                        nc.vector.tensor_copy(
                            V_b[:, base + g * NW:base + (g + 1) * NW], vtp[:])

                    vt_pending = None
                    for g in range(NG):
                        gg = (b * S + base) // NW + g   # global 256-tok group
                        qab = pps.tile([128, 2 * NW], F32, tag="qab", bufs=2)
                        qcd = pps.tile([128, 2 * NW], F32, tag="qcd", bufs=2)
                        kv = pps.tile([128, 2 * NW], F32, tag="kv", bufs=1)
                        for jb in range(NJB):
                            xg = xpool.tile([128, 8 * NW], BF16, tag="xg")
                            r0 = (gg * NJB + jb) * 128
                            if first_half and g == 0 and jb == 0:
                                # staged startup: load the first x block in
                                # halves so the very first matmul gates on
                                # ~1MB of DMA, not ~6MB
                                nc.sync.dma_start(xg[:, 0:4 * NW],
                                                  x4[r0:r0 + 128, 0:4 * NW])
                                nc.sync.dma_start(
                                    wq_sb[:, 4 * FL:8 * FL],
                                    wq4[:, 4 * FL:8 * FL])
                                nc.sync.dma_start(xg[:, 4 * NW:8 * NW],
                                                  x4[r0:r0 + 128, 4 * NW:8 * NW])
                                nc.sync.dma_start(wk_sb[:], wk4)
                                nc.sync.dma_start(wv_sb[:], wv4)
                            else:
                                nc.sync.dma_start(xg[:], x4[r0:r0 + 128, :])
                            if first_half and g == 0 and jb == 1:
                                emit_late_weight_loads()
                            if first_half and g == 0 and jb == 2:
                                # rope tables only needed at this group's
                                # evictions; keep the first x tile's DMA
                                # bandwidth share high
                                nc.sync.dma_start(
                                    cos_s[:], ropc[:, base:base + HALF])
                                nc.sync.dma_start(
                                    sin_s[:], rops[:, base:base + HALF])
                            # q matmuls first, kv last: the kv bank is
                            # single-buffered, so its previous-group RoPE /
                            # copy reads get ~3.4us of q-matmul cover before
                            # the next write touches the bank.
                            for c in range(8):
                                k = jb * 8 + c
                                xs = xg[:, c * NW:(c + 1) * NW]
                                st = (k == 0)
                                sp = (k == DKD - 1)
                                nc.tensor.matmul(
                                    qab[:, 0:NW],
                                    wq_sb[:, k * FL:k * FL + 128], xs,
                                    start=st, stop=False)
                                nc.tensor.matmul(
                                    qab[:, NW:2 * NW],
                                    wq_sb[:, k * FL + 128:k * FL + 256], xs,
                                    start=False, stop=sp)
                                nc.tensor.matmul(
                                    qcd[:, 0:NW],
                                    wq_sb[:, k * FL + 256:k * FL + 384], xs,
                                    start=st, stop=False)
                                nc.tensor.matmul(
                                    qcd[:, NW:2 * NW],
                                    wq_sb[:, k * FL + 384:k * FL + 512], xs,
                                    start=False, stop=sp)
                            for c in range(8):
                                k = jb * 8 + c
                                xs = xg[:, c * NW:(c + 1) * NW]
                                st = (k == 0)
                                sp = (k == DKD - 1)
                                nc.tensor.matmul(
                                    kv[:, 0:NW],
                                    wk_sb[:, k * HD:(k + 1) * HD], xs,
                                    start=st, stop=False)
                                nc.tensor.matmul(
                                    kv[:, NW:2 * NW],
                                    wv_sb[:, k * HD:(k + 1) * HD], xs,
                                    start=False, stop=sp)
                        # evictions: k rope + v copy first (kv is bufs=1) so
                        # the next group's kv matmuls aren't gated on them —
                        # except for the LAST group, where the q ropes go
                        # first: the attention phase reuses these PSUM banks
                        # and its first score matmuls wait on the q reads.
                        vst = vstpool.tile([128, NW], F32R, tag="vst")
                        last_g = (g == NG - 1)
                        if not last_g:
                            rope_evict(kv[:, 0:NW],
                                       kT[:, base + g * NW:base + (g + 1) * NW],
                                       g)
                            nc.scalar.copy(vst[:], kv[:, NW:2 * NW])
                        # previous group's V transpose here: its DVE copy
                        # overlaps this group's matmuls (vtp is bufs=1)
                        if vt_pending is not None:
                            emit_vt(*vt_pending)
                        vt_pending = (g, vst)
                        rope_evict(qab[:, 0:NW],
                                   q_half[0][:, g * NW:(g + 1) * NW], g)
                        rope_evict(qab[:, NW:2 * NW],
                                   q_half[1][:, g * NW:(g + 1) * NW], g)
                        rope_evict(qcd[:, 0:NW],
                                   q_half[2][:, g * NW:(g + 1) * NW], g)
                        rope_evict(qcd[:, NW:2 * NW],
                                   q_half[3][:, g * NW:(g + 1) * NW], g)
                        if last_g:
                            rope_evict(kv[:, 0:NW],
                                       kT[:, base + g * NW:base + (g + 1) * NW],
                                       g)
                            nc.scalar.copy(vst[:], kv[:, NW:2 * NW])
                        if b == 0 and hb == 0 and g == 0:
                            emit_const_loads()
                    emit_vt(*vt_pending)

                # ----------------------------------------------------------
                # Attention for this half (q blocks of 512)
                # ----------------------------------------------------------
                att_h = [attpool.tile([128, HALF], F32R, tag=f"at{f}",
                                      name=f"at{f}") for f in range(QH)]
                with tc.tile_pool(name="attnps", bufs=1, space="PSUM") as aps:
                    for qb in range(HALF // QB):
                        q0 = qb * QB
                        nkt = (base + q0 + QB) // 128
                        for h in range(QH):
                            avp = aps.tile([128, QB], F32, tag="avp", bufs=3)
                            smp = aps.tile([1, QB], F32, tag="smp", bufs=1)
                            acc = accpool.tile([128, QB], F32R, tag="acc")
                            for ktc in range(nkt):
                                stp = aps.tile([128, QB], F32, tag="stp",
                                               bufs=2)
                                diag = ktc >= nkt - 4
                                nc.tensor.matmul(
                                    stp[:], kT[:, ktc * 128:(ktc + 1) * 128],
                                    q_half[h][:, q0:q0 + QB],
                                    start=True, stop=not diag)
                                if diag:
                                    # causal mask added on the PE: I.T @ M
                                    # accumulates M into the score bank,
                                    # keeping the chunk chain PE->ACT only.
                                    # chunk v only masks q columns < (v+1)*128
                                    # (the rest of the mask slice is zeros)
                                    v = ktc - (nkt - 4)
                                    mw = (v + 1) * 128
                                    nc.tensor.matmul(
                                        stp[:, 0:mw], ident[:],
                                        mtv[v][:, 0:mw],
                                        start=False, stop=True)
                                pt = ptpool.tile([128, QB], F32R, tag="pt")
                                nc.scalar.activation(pt[:], stp[:], EXP,
                                                     scale=SCALE)
                                nc.tensor.matmul(
                                    avp[:], V_b[:, ktc * 128:(ktc + 1) * 128],
                                    pt[:], start=(ktc == 0),
                                    stop=(ktc == nkt - 1))
                                # exp tiles accumulate on DVE; one ones-matmul
                                # per head then reduces over partitions (vs a
                                # third matmul on every chunk)
                                if ktc == 0:
                                    nc.vector.tensor_copy(acc[:], pt[:])
                                else:
                                    nc.vector.tensor_add(acc[:], acc[:], pt[:])
                            nc.tensor.matmul(smp[:], ones_t[:, 0:1], acc[:],
                                             start=True, stop=True)
                            # evictions alternate ACT/DVE by head so a
                            # backlog on either queue can't stall the avp
                            # bank rotation two heads later
                            att_un = aupool.tile([128, QB], F32R, tag="au")
                            s_sb = smpool.tile([1, QB], F32, tag="ssb")
                            if h % 2 == 0:
                                nc.scalar.copy(att_un[:], avp[:])
                                nc.vector.tensor_copy(s_sb[:], smp[:])
                            else:
                                nc.vector.tensor_copy(att_un[:], avp[:])
                                nc.scalar.copy(s_sb[:], smp[:])
                            r_sb = smpool.tile([1, QB], F32, tag="rsb")
                            nc.vector.reciprocal_approx_fast(r_sb[:], s_sb[:])
                            r_bc = accpool.tile([128, QB], F32, tag="rbc")
                            nc.gpsimd.partition_broadcast(r_bc[:], r_sb[:])
                            nc.vector.tensor_mul(
                                att_h[h][:, q0:q0 + QB], att_un[:], r_bc[:])
                # previous half's output projection: emitted after this
                # half's attention; the scheduler interleaves its PE work
                # into attention's dependency stalls
                if pending is not None:
                    emit_wo(*pending)
                pending = (att_h, b, hb)
        emit_wo(*pending)
    nc.compile()
    return nc


_program = None


def _get_program():
    global _program
    if _program is None:
        _program = _build_program()
    return _program


def kernel(**inputs) -> np.ndarray:
    x = np.asarray(inputs["x"], dtype=np.float32)
    wq = np.asarray(inputs["wq"], dtype=np.float32)
    wk = np.asarray(inputs["wk"], dtype=np.float32)
    wv = np.asarray(inputs["wv"], dtype=np.float32)
    wo = np.asarray(inputs["wo"], dtype=np.float32)
    cos = np.asarray(inputs["freqs_cos"], dtype=np.float32)
    sin = np.asarray(inputs["freqs_sin"], dtype=np.float32)
    mask = np.asarray(inputs["mask"], dtype=np.float32)
    start_pos = int(np.asarray(inputs.get("start_pos", 0)))
    assert start_pos == 0, "kernel specialized for start_pos == 0"

    # Even/odd RoPE pair split within each head's 128 features.
    perm = np.concatenate([np.arange(0, HD, 2), np.arange(1, HD, 2)])

    # x tiled: x4[gg, jb] rows = [128, 8*256] where row p, col c*256+w =
    # x_token[gg*256 + w, (jb*8+c)*128 + p]
    xT = x.reshape(T, D).T                              # [D, T]
    x4 = np.ascontiguousarray(
        xT.reshape(NJB, 8, 128, 16, NW).transpose(3, 0, 2, 1, 4)
        .reshape(16 * NJB * 128, 8 * NW)).astype(ml_dtypes.bfloat16)

    cosT = cos.T                                        # [64, S]
    sinT = sin.T
    ropc = np.ascontiguousarray(np.concatenate([cosT, cosT], axis=0))
    rops = np.ascontiguousarray(np.concatenate([-sinT, sinT], axis=0))
    masktv = [np.ascontiguousarray(
        np.maximum(mask[:QB, v * 128:(v + 1) * 128], NEG)
        .astype(np.float32).T) for v in range(4)]

    in_maps = []
    for c in range(N_CORES):
        wq_c = (wq[c * FL:(c + 1) * FL].reshape(QH, HD, D)[:, perm, :]
                .reshape(FL, D))
        wk_c = wk[c * HD:(c + 1) * HD][perm, :]
        wv_c = wv[c * HD:(c + 1) * HD]
        wo_c = wo[:, c * FL:(c + 1) * FL]
        # wq4[p, k*512 + f] = wq_c[f, k*128+p]  (k-chunk-major, bf16)
        wq4 = np.ascontiguousarray(
            wq_c.T.reshape(DKD, 128, FL).transpose(1, 0, 2)
            .reshape(128, DKD * FL)).astype(ml_dtypes.bfloat16)
        wk4 = np.ascontiguousarray(
            wk_c.T.reshape(DKD, 128, HD).transpose(1, 0, 2)
            .reshape(128, DKD * HD)).astype(ml_dtypes.bfloat16)
        wv4 = np.ascontiguousarray(
            wv_c.T.reshape(DKD, 128, HD).transpose(1, 0, 2)
            .reshape(128, DKD * HD)).astype(ml_dtypes.bfloat16)
        # wot[dg*128+p, f*512+c] = wo_c[dg*512+c, f*128+p]
        wot = np.ascontiguousarray(
            wo_c.T.reshape(QH, 128, NDG, QB).transpose(2, 1, 0, 3)
            .reshape(NDG * 128, QH * QB))
        in_maps.append({
            "x4": x4,
            "wq4": wq4,
            "wk4": wk4,
            "wv4": wv4,
            "wot": wot,
            "ropc": ropc,
            "rops": rops,
            "idin": np.eye(128, dtype=np.float32),
            "onesin": np.ones((128, 1), dtype=np.float32),
            "maskt0": masktv[0],
            "maskt1": masktv[1],
            "maskt2": masktv[2],
            "maskt3": masktv[3],
        })

    nc = _get_program()
    trace = bool(int(os.environ.get("GQA_TRACE", "0")))
    kwargs = {}
    if trace:
        tmpdir = os.environ.get("GQA_TRACE_DIR") or None
        kwargs = dict(trace=True, tmpdir=tmpdir, trace_cores=[0])
    res = run_bass_kernel_spmd(nc, in_maps, list(range(N_CORES)), **kwargs)
    kernel.last_results = res

    acc = np.zeros((T // 128, 128, D), dtype=np.float64)
    for c in range(N_CORES):
        yt = np.asarray(res.results[c]["y_t"], dtype=np.float64)
        acc += yt.reshape(T // 128, NDG, 128, QB).transpose(0, 2, 1, 3) \
                 .reshape(T // 128, 128, D)
    return acc.astype(np.float32).reshape(B, S, D)
